# revision 8
# baseline (speedup 1.0000x reference)
"""Adaptive polyphase sampling (stride 2, p=2) on 8 TRN2 NeuronCores.

For x [32, 256, 64, 64] f32: compute the 4 polyphase components
x[:, :, i::2, j::2], pick per-sample the component with the largest L2
norm (over channels+space), return it [32, 256, 32, 32].

Sharding: pure data parallel over batch — 4 samples per core, no
cross-core communication.

Layout: partition p holds the channel pair {2p, 2p+1} so each
partition's DMA slice is one contiguous 32 KiB run (best descriptor
shape both directions).

Per-core dataflow (samples s = 0..3):
  sync   : DMA x[s] -> samp[s%3], issue serialized so completions are
           in order and sample 0 lands as early as possible
  scalar : norms k=0,1,2 (Square activation + accum_out)
           t_a = V3 * m3                       (Copy act, scale AP)
  vector : norm k=3 (scalar_tensor_tensor square + accum_out)
           mask: reduce_max(psum) + is_equal -> mask[:, 4s:4s+4]
           t_b = V2*m2 ; c_b = (V1*m1)+t_b ; c_a = (V0*m0)+t_a  (STT)
  tensor : ones[128,128] @ norms[:, 4s:4s+4] -> psum (channel reduce +
           broadcast of per-sample component norms to all partitions)
  gpsimd : memset ones; O = c_a + c_b -> obuf[s%2]; DMA obuf -> out[s]

The argmax is realized as mask_k = (norm_k == max_k norm_k) in {0,1},
then O = sum_k mask_k * V_k. Exact float ties between component norms
(sums of ~1M random squares) are probability-zero.

Synchronization: engines are pipelined, so same-engine data deps need
semaphore handshakes (writes land at DRAIN). Every compute op on
scalar/vector/gpsimd increments its engine chain sem (sch/vch/gch)
and waits for all previously-emitted ops on that engine; cross-engine
waits reference chain thresholds from the static emission plan below.
DMA completions are unordered across transactions, so each in-flight
DMA direction/slot gets its own semaphore.
"""

from contextlib import ExitStack

import numpy as np

import concourse.bass as bass
from concourse import mybir
from concourse.bass_utils import run_bass_kernel_spmd

F32 = mybir.dt.float32
AX = mybir.AxisListType
OP = mybir.AluOpType
ACT = mybir.ActivationFunctionType

B, C, H, W = 32, 256, 64, 64
NCORES = 8
SPC = B // NCORES          # samples per core
H2, W2 = H // 2, W // 2    # 32, 32
SP = H * W                 # 4096 spatial elems per (sample, channel)
OSP = H2 * W2              # 1024

N_SAMP_BUFS = 3
N_OBUFS = 2

# ---- static emission plan (op counts per group) --------------------------
# scalar: n = norms k=0,1,2 ; sel = t_a
SC_SIZES = {"n": 3, "sel": 1}
SC_ORDER = [("n", 0), ("n", 1), ("sel", 0), ("n", 2), ("sel", 1),
            ("n", 3), ("sel", 2), ("sel", 3)]
# vector: vn = norm k=3 ; mk = reduce_max + is_equal ; vs = t_b, c_b, c_a
VE_SIZES = {"vn": 1, "mk": 2, "vs": 3}
VE_ORDER = [("vn", 0), ("vn", 1), ("mk", 0), ("vs", 0), ("mk", 1),
            ("vs", 1), ("vn", 2), ("mk", 2), ("vs", 2), ("vn", 3),
            ("mk", 3), ("vs", 3)]
# gpsimd: one O-add per sample (plus the initial memset, counted separately)
GP_SIZES = {"o": 1}
GP_ORDER = [("o", s) for s in range(SPC)]


def _plan(order, sizes, base=0):
    done, start, c = {}, {}, base
    for g, s in order:
        start[(g, s)] = c
        c += sizes[g]
        done[(g, s)] = c
    return done, start, c


SC_DONE, SC_START, SC_TOTAL = _plan(SC_ORDER, SC_SIZES)
VE_DONE, VE_START, VE_TOTAL = _plan(VE_ORDER, VE_SIZES)
GP_DONE, GP_START, GP_TOTAL = _plan(GP_ORDER, GP_SIZES, base=1)  # memset is op 1


def _comp_view(samp_ap, k):
    """[128, 2, 32, 32] strided view of polyphase component k=(i,j)."""
    i, j = divmod(k, 2)
    v6 = samp_ap.rearrange("p c (r i q j) -> p c r i q j", r=H2, i=2, q=W2, j=2)
    return v6[:, :, :, i, :, j]


def build_nc():
    nc = bass.Bass("TRN2", target_bir_lowering=False, debug=False)
    x = nc.dram_tensor("x", [SPC, C, H, W], F32, kind="ExternalInput")
    out = nc.dram_tensor("out", [SPC, C, H2, W2], F32, kind="ExternalOutput")

    # partition p <- channel pair {2p, 2p+1}: contiguous 32 KiB per partition
    x_aps = [
        x.ap()[s].rearrange("(p c) h w -> p c (h w)", c=2) for s in range(SPC)
    ]
    out_aps = [
        out.ap()[s].rearrange("(p c) a b -> p c (a b)", c=2) for s in range(SPC)
    ]

    with ExitStack() as ctx:
        block = ctx.enter_context(nc.Block())
        sem = lambda name: ctx.enter_context(nc.semaphore(name))
        sb = lambda name, shape: ctx.enter_context(nc.sbuf_tensor(name, shape, F32))
        dmains = [sem(f"dmain{s}") for s in range(SPC)]
        dmaouts = [sem(f"dmaout{i}") for i in range(N_OBUFS)]
        sch, vch, gch, mm = sem("sch"), sem("vch"), sem("gch"), sem("mm")
        samps = [sb(f"samp{i}", [128, 2, SP]) for i in range(N_SAMP_BUFS)]
        obufs = [sb(f"obuf{i}", [128, 2, OSP]) for i in range(N_OBUFS)]
        ta = sb("ta", [128, 2, OSP])
        tb = sb("tb", [128, 2, OSP])
        ca = sb("ca", [128, 2, OSP])
        cb = sb("cb", [128, 2, OSP])
        sqs = sb("sqs", [128, 2, OSP])
        sqv = sb("sqv", [128, 2, OSP])
        norms = sb("norms", [128, 4 * SPC])
        mask = sb("mask", [128, 4 * SPC])
        mx = sb("mx", [128, SPC])
        ones = sb("ones", [128, 128])
        psums = [
            ctx.enter_context(nc.psum_tensor(f"ps{i}", [128, 4], F32))
            for i in range(2)
        ]

        def V(s, k):
            return _comp_view(samps[s % N_SAMP_BUFS].ap(), k)

        sq_view = lambda t: t.ap().rearrange("p c (r q) -> p c r q", r=H2)
        ncol = lambda s, k: norms.ap()[:, 4 * s + k : 4 * s + k + 1]
        mcol = lambda s, k: mask.ap()[:, 4 * s + k : 4 * s + k + 1]

        @block.sync
        def _(sync):
            for s in range(SPC):
                if s >= 1:
                    # serialize issues: completions in order, sample 0 early
                    sync.wait_ge(dmains[s - 1], 16)
                if s >= N_SAMP_BUFS:
                    # samp[s%3] reuse: all consumers of sample s-3 done
                    sp = s - N_SAMP_BUFS
                    sync.wait_ge(sch, SC_DONE[("sel", sp)])
                    sync.wait_ge(vch, VE_DONE[("vs", sp)])
                sync.dma_start(out=samps[s % N_SAMP_BUFS].ap(), in_=x_aps[s]).then_inc(
                    dmains[s], 16
                )

        @block.tensor
        def _(tensor):
            tensor.wait_ge(gch, 1)  # ones ready
            for s in range(SPC):
                tensor.wait_ge(sch, SC_DONE[("n", s)])
                tensor.wait_ge(vch, VE_DONE[("vn", s)])
                if s >= 2:
                    # psum[s%2] reuse: mask ops of sample s-2 done reading
                    tensor.wait_ge(vch, VE_DONE[("mk", s - 2)])
                tensor.matmul(
                    psums[s % 2].ap(),
                    ones.ap(),
                    norms.ap()[:, 4 * s : 4 * s + 4],
                    start=True,
                    stop=True,
                ).then_inc(mm, 1)

        @block.scalar
        def _(scalar):
            cnt = [0]

            def emit(inst):
                inst.then_inc(sch, 1)
                cnt[0] += 1

            def barrier():
                if cnt[0]:
                    scalar.wait_ge(sch, cnt[0])

            def n(s):
                scalar.wait_ge(dmains[s], 16)
                for k in (0, 1, 2):
                    barrier()
                    emit(
                        scalar.activation(
                            sq_view(sqs), V(s, k), ACT.Square, accum_out=ncol(s, k)
                        )
                    )

            def selp(s):
                scalar.wait_ge(vch, VE_DONE[("mk", s)])
                if s >= 1:
                    # ta reuse: c_a of sample s-1 consumed it
                    scalar.wait_ge(vch, VE_DONE[("vs", s - 1)])
                barrier()
                emit(
                    scalar.activation(
                        sq_view(ta), V(s, 3), ACT.Copy, scale=mcol(s, 3)
                    )
                )

            for g, s in SC_ORDER:
                n(s) if g == "n" else selp(s)
            assert cnt[0] == SC_TOTAL

        @block.vector
        def _(vector):
            cnt = [0]

            def emit(inst):
                inst.then_inc(vch, 1)
                cnt[0] += 1

            def barrier():
                if cnt[0]:
                    vector.wait_ge(vch, cnt[0])

            def vn(s):
                vector.wait_ge(dmains[s], 16)
                barrier()
                emit(
                    vector.scalar_tensor_tensor(
                        out=sq_view(sqv),
                        in0=V(s, 3),
                        scalar=0.0,
                        in1=V(s, 3),
                        op0=OP.bypass,
                        op1=OP.mult,
                        accum_out=ncol(s, 3),
                    )
                )

            def mk(s):
                vector.wait_ge(mm, s + 1)
                barrier()
                emit(
                    vector.reduce_max(
                        mx.ap()[:, s : s + 1], psums[s % 2].ap(), axis=AX.X
                    )
                )
                barrier()
                emit(
                    vector.tensor_scalar(
                        out=mask.ap()[:, 4 * s : 4 * s + 4],
                        in0=psums[s % 2].ap(),
                        scalar1=mx.ap()[:, s : s + 1],
                        scalar2=None,
                        op0=OP.is_equal,
                    )
                )

            def vs(s):
                # tb/cb/ca reuse: O-add of sample s-1 consumed them
                if s >= 1:
                    vector.wait_ge(gch, GP_DONE[("o", s - 1)])
                barrier()
                emit(vector.tensor_scalar_mul(sq_view(tb), V(s, 2), mcol(s, 2)))
                barrier()
                emit(
                    vector.scalar_tensor_tensor(
                        out=cb.ap(),
                        in0=V(s, 1),
                        scalar=mcol(s, 1),
                        in1=tb.ap(),
                        op0=OP.mult,
                        op1=OP.add,
                    )
                )
                vector.wait_ge(sch, SC_DONE[("sel", s)])
                barrier()
                emit(
                    vector.scalar_tensor_tensor(
                        out=ca.ap(),
                        in0=V(s, 0),
                        scalar=mcol(s, 0),
                        in1=ta.ap(),
                        op0=OP.mult,
                        op1=OP.add,
                    )
                )

            fns = {"vn": vn, "mk": mk, "vs": vs}
            for g, s in VE_ORDER:
                fns[g](s)
            assert cnt[0] == VE_TOTAL

        @block.gpsimd
        def _(gpsimd):
            gpsimd.memset(ones.ap(), 1.0).then_inc(gch, 1)
            gcnt = 1
            for s in range(SPC):
                gpsimd.wait_ge(vch, VE_DONE[("vs", s)])
                if s >= N_OBUFS:
                    # obuf[s%2] reuse: out-DMA of sample s-2 done reading
                    gpsimd.wait_ge(dmaouts[s % N_OBUFS], 16 * (s // N_OBUFS))
                gpsimd.wait_ge(gch, gcnt)
                gpsimd.tensor_add(
                    obufs[s % N_OBUFS].ap(), ca.ap(), cb.ap()
                ).then_inc(gch, 1)
                gcnt += 1
                assert gcnt == GP_DONE[("o", s)]
                gpsimd.wait_ge(gch, gcnt)
                gpsimd.dma_start(out=out_aps[s], in_=obufs[s % N_OBUFS].ap()).then_inc(
                    dmaouts[s % N_OBUFS], 16
                )

    return nc


_NC_CACHE = None


def _get_nc():
    global _NC_CACHE
    if _NC_CACHE is None:
        _NC_CACHE = build_nc()
    return _NC_CACHE


def kernel(x) -> np.ndarray:
    x = np.asarray(x, dtype=np.float32)
    assert x.shape == (B, C, H, W), x.shape
    shards = np.split(x, NCORES, axis=0)
    in_maps = [{"x": s} for s in shards]
    res = run_bass_kernel_spmd(_get_nc(), in_maps, core_ids=list(range(NCORES)))
    return np.concatenate([r["out"] for r in res.results], axis=0)


# revision 9
# speedup vs baseline: 1.2420x; 1.2420x over previous
"""Adaptive polyphase sampling (stride 2, p=2) on 8 TRN2 NeuronCores.

For x [32, 256, 64, 64] f32: compute the 4 polyphase components
x[:, :, i::2, j::2], pick per-sample the component with the largest L2
norm (over channels+space), return it [32, 256, 32, 32].

Sharding: pure data parallel over batch — 4 samples per core, no
cross-core communication.

Layout: partition p holds the channel pair {2p, 2p+1}; each channel
plane is a contiguous 16 KiB run per partition.

DMA: one HWDGE stream tops out at ~270 GB/s, so input runs on TWO
concurrent streams — sync (SP/HWDGE) carries even-channel planes,
gpsimd (SWDGE) carries odd-channel planes — each serialized per
sample so completions arrive in order (~12 us cadence).

Per-core dataflow (samples s = 0..3):
  sync   : DMA x[s] even channels -> samp[s%3][:, 0]
  gpsimd : memset ones; DMA x[s] odd channels -> samp[s%3][:, 1];
           DMA obuf[s%2] -> out[s]
  scalar : norms k=0..3 (Square activation + accum_out); sample 3
           does only k=0,1 (k=2,3 go to vector to shorten the tail)
  vector : mask: reduce_max(psum) + is_equal -> mask[:, 4s:4s+4]
           select: c = V0*m0; c = (V1*m1)+c; c = (V2*m2)+c;
                   obuf = (V3*m3)+c          (scalar_tensor_tensor)
  tensor : ones[128,128] @ norms[:, 4s:4s+4] -> psum (channel reduce
           + broadcast of component norms to all partitions)

The argmax is realized as mask_k = (norm_k == max_k norm_k) in {0,1},
then O = sum_k mask_k * V_k. Exact float ties between component norms
(sums of ~1M random squares) are probability-zero.

Synchronization: engines are pipelined, so same-engine data deps need
semaphore handshakes (writes land at DRAIN). Every compute op on
scalar/vector increments its engine chain sem (sch/vch) and waits for
all previously-emitted ops on that engine; cross-engine waits
reference chain thresholds from the static emission plan below. DMA
completions are unordered across transactions, so every in-flight DMA
gets its own semaphore.
"""

from contextlib import ExitStack

import numpy as np

import concourse.bass as bass
from concourse import mybir
from concourse.bass_utils import run_bass_kernel_spmd

F32 = mybir.dt.float32
AX = mybir.AxisListType
OP = mybir.AluOpType
ACT = mybir.ActivationFunctionType

B, C, H, W = 32, 256, 64, 64
NCORES = 8
SPC = B // NCORES          # samples per core
H2, W2 = H // 2, W // 2    # 32, 32
SP = H * W                 # 4096 spatial elems per channel
OSP = H2 * W2              # 1024

N_SAMP_BUFS = 3
N_OBUFS = 2
LAST = SPC - 1

# ---- static emission plan ------------------------------------------------
# scalar: 4 Square+accum per sample, except sample LAST has 2 (k=0,1)
SC_SIZES = {("n", s): (2 if s == LAST else 4) for s in range(SPC)}
SC_ORDER = [("n", s) for s in range(SPC)]
# vector: vn = 2 STT-norms (only sample LAST) ; mk = 2 ; ch = 4 chain ops
VE_ORDER = [("mk", 0), ("ch", 0), ("mk", 1), ("ch", 1), ("mk", 2),
            ("ch", 2), ("vn", LAST), ("mk", 3), ("ch", 3)]
VE_SIZES_G = {"vn": 2, "mk": 2, "ch": 4}


def _plan(order, sizes):
    done, start, c = {}, {}, 0
    for key in order:
        start[key] = c
        c += sizes[key] if key in sizes else sizes[key[0]]
        done[key] = c
    return done, start, c


SC_DONE, SC_START, SC_TOTAL = _plan(SC_ORDER, SC_SIZES)
VE_DONE, VE_START, VE_TOTAL = _plan(VE_ORDER, VE_SIZES_G)


def _comp_view(samp_ap, k):
    """[128, 2, 32, 32] strided view of polyphase component k=(i,j)."""
    i, j = divmod(k, 2)
    v6 = samp_ap.rearrange("p c (r i q j) -> p c r i q j", r=H2, i=2, q=W2, j=2)
    return v6[:, :, :, i, :, j]


def build_nc():
    nc = bass.Bass("TRN2", target_bir_lowering=False, debug=False)
    x = nc.dram_tensor("x", [SPC, C, H, W], F32, kind="ExternalInput")
    out = nc.dram_tensor("out", [SPC, C, H2, W2], F32, kind="ExternalOutput")

    # channel planes: plane h of sample s = channels c with c%2 == h
    def x_plane(s, h):
        # [128, 4096]: partition p <- channel 2p+h, contiguous 16 KiB run
        return x.ap()[s].rearrange("(p c) h w -> p c (h w)", c=2)[:, h]

    out_aps = [
        out.ap()[s].rearrange("(p c) a b -> p c (a b)", c=2) for s in range(SPC)
    ]

    with ExitStack() as ctx:
        block = ctx.enter_context(nc.Block())
        sem = lambda name: ctx.enter_context(nc.semaphore(name))
        sb = lambda name, shape: ctx.enter_context(nc.sbuf_tensor(name, shape, F32))
        dmA = [sem(f"dmA{s}") for s in range(SPC)]
        dmB = [sem(f"dmB{s}") for s in range(SPC)]
        dmaouts = [sem(f"dmaout{i}") for i in range(N_OBUFS)]
        g1, sch, vch, mm = sem("g1"), sem("sch"), sem("vch"), sem("mm")
        samps = [sb(f"samp{i}", [128, 2, SP]) for i in range(N_SAMP_BUFS)]
        obufs = [sb(f"obuf{i}", [128, 2, OSP]) for i in range(N_OBUFS)]
        cb1 = sb("cb1", [128, 2, OSP])
        cb2 = sb("cb2", [128, 2, OSP])
        sqs = sb("sqs", [128, 2, OSP])
        sqv = sb("sqv", [128, 2, OSP])
        norms = sb("norms", [128, 4 * SPC])
        mask = sb("mask", [128, 4 * SPC])
        mx = sb("mx", [128, SPC])
        ones = sb("ones", [128, 128])
        psums = [
            ctx.enter_context(nc.psum_tensor(f"ps{i}", [128, 4], F32))
            for i in range(2)
        ]

        def V(s, k):
            return _comp_view(samps[s % N_SAMP_BUFS].ap(), k)

        sq_view = lambda t: t.ap().rearrange("p c (r q) -> p c r q", r=H2)
        ncol = lambda s, k: norms.ap()[:, 4 * s + k : 4 * s + k + 1]
        mcol = lambda s, k: mask.ap()[:, 4 * s + k : 4 * s + k + 1]

        def samp_war_waits(eng, s):
            """Before DMA-writing samp[s%3]: consumers of sample s-3 done."""
            if s >= N_SAMP_BUFS:
                sp = s - N_SAMP_BUFS
                eng.wait_ge(sch, SC_DONE[("n", sp)])
                eng.wait_ge(vch, VE_DONE[("ch", sp)])

        @block.sync
        def _(sync):
            for s in range(SPC):
                if s >= 1:
                    sync.wait_ge(dmA[s - 1], 16)
                samp_war_waits(sync, s)
                sync.dma_start(
                    out=samps[s % N_SAMP_BUFS].ap()[:, 0], in_=x_plane(s, 0)
                ).then_inc(dmA[s], 16)

        @block.gpsimd
        def _(gpsimd):
            gpsimd.memset(ones.ap(), 1.0).then_inc(g1, 1)

            def h1(s):
                if s >= 1:
                    gpsimd.wait_ge(dmB[s - 1], 16)
                samp_war_waits(gpsimd, s)
                gpsimd.dma_start(
                    out=samps[s % N_SAMP_BUFS].ap()[:, 1], in_=x_plane(s, 1)
                ).then_inc(dmB[s], 16)

            def outd(s):
                gpsimd.wait_ge(vch, VE_DONE[("ch", s)])
                gpsimd.dma_start(
                    out=out_aps[s], in_=obufs[s % N_OBUFS].ap()
                ).then_inc(dmaouts[s % N_OBUFS], 16)

            h1(0)
            h1(1)
            h1(2)
            outd(0)
            h1(3)
            outd(1)
            outd(2)
            outd(3)

        @block.tensor
        def _(tensor):
            tensor.wait_ge(g1, 1)  # ones ready
            for s in range(SPC):
                tensor.wait_ge(sch, SC_DONE[("n", s)])
                if s == LAST:
                    tensor.wait_ge(vch, VE_DONE[("vn", LAST)])
                if s >= 2:
                    # psum[s%2] reuse: mask ops of sample s-2 done reading
                    tensor.wait_ge(vch, VE_DONE[("mk", s - 2)])
                tensor.matmul(
                    psums[s % 2].ap(),
                    ones.ap(),
                    norms.ap()[:, 4 * s : 4 * s + 4],
                    start=True,
                    stop=True,
                ).then_inc(mm, 1)

        @block.scalar
        def _(scalar):
            cnt = [0]

            def emit(inst):
                inst.then_inc(sch, 1)
                cnt[0] += 1

            def barrier():
                if cnt[0]:
                    scalar.wait_ge(sch, cnt[0])

            for s in range(SPC):
                scalar.wait_ge(dmA[s], 16)
                scalar.wait_ge(dmB[s], 16)
                ks = (0, 1) if s == LAST else (0, 1, 2, 3)
                for k in ks:
                    barrier()
                    emit(
                        scalar.activation(
                            sq_view(sqs), V(s, k), ACT.Square, accum_out=ncol(s, k)
                        )
                    )
            assert cnt[0] == SC_TOTAL

        @block.vector
        def _(vector):
            cnt = [0]

            def emit(inst):
                inst.then_inc(vch, 1)
                cnt[0] += 1

            def barrier():
                if cnt[0]:
                    vector.wait_ge(vch, cnt[0])

            def vn(s):
                vector.wait_ge(dmA[s], 16)
                vector.wait_ge(dmB[s], 16)
                for k in (2, 3):
                    barrier()
                    emit(
                        vector.scalar_tensor_tensor(
                            out=sq_view(sqv),
                            in0=V(s, k),
                            scalar=0.0,
                            in1=V(s, k),
                            op0=OP.bypass,
                            op1=OP.mult,
                            accum_out=ncol(s, k),
                        )
                    )

            def mk(s):
                vector.wait_ge(mm, s + 1)
                barrier()
                emit(
                    vector.reduce_max(
                        mx.ap()[:, s : s + 1], psums[s % 2].ap(), axis=AX.X
                    )
                )
                barrier()
                emit(
                    vector.tensor_scalar(
                        out=mask.ap()[:, 4 * s : 4 * s + 4],
                        in0=psums[s % 2].ap(),
                        scalar1=mx.ap()[:, s : s + 1],
                        scalar2=None,
                        op0=OP.is_equal,
                    )
                )

            def ch(s):
                barrier()
                emit(vector.tensor_scalar_mul(sq_view(cb1), V(s, 0), mcol(s, 0)))
                barrier()
                emit(
                    vector.scalar_tensor_tensor(
                        out=cb2.ap(), in0=V(s, 1), scalar=mcol(s, 1), in1=cb1.ap(),
                        op0=OP.mult, op1=OP.add,
                    )
                )
                barrier()
                emit(
                    vector.scalar_tensor_tensor(
                        out=cb1.ap(), in0=V(s, 2), scalar=mcol(s, 2), in1=cb2.ap(),
                        op0=OP.mult, op1=OP.add,
                    )
                )
                if s >= N_OBUFS:
                    # obuf[s%2] reuse: out-DMA of sample s-2 done reading
                    vector.wait_ge(dmaouts[s % N_OBUFS], 16 * (s // N_OBUFS))
                barrier()
                emit(
                    vector.scalar_tensor_tensor(
                        out=obufs[s % N_OBUFS].ap(), in0=V(s, 3), scalar=mcol(s, 3),
                        in1=cb1.ap(), op0=OP.mult, op1=OP.add,
                    )
                )

            fns = {"vn": vn, "mk": mk, "ch": ch}
            for g, s in VE_ORDER:
                fns[g](s)
            assert cnt[0] == VE_TOTAL

    return nc


_NC_CACHE = None


def _get_nc():
    global _NC_CACHE
    if _NC_CACHE is None:
        _NC_CACHE = build_nc()
    return _NC_CACHE


def kernel(x) -> np.ndarray:
    x = np.asarray(x, dtype=np.float32)
    assert x.shape == (B, C, H, W), x.shape
    shards = np.split(x, NCORES, axis=0)
    in_maps = [{"x": s} for s in shards]
    res = run_bass_kernel_spmd(_get_nc(), in_maps, core_ids=list(range(NCORES)))
    return np.concatenate([r["out"] for r in res.results], axis=0)


# revision 12
# speedup vs baseline: 1.3109x; 1.0555x over previous
"""Adaptive polyphase sampling (stride 2, p=2) on 8 TRN2 NeuronCores.

For x [32, 256, 64, 64] f32: compute the 4 polyphase components
x[:, :, i::2, j::2], pick per-sample the component with the largest L2
norm (over channels+space), return it [32, 256, 32, 32].

Sharding: pure data parallel over batch — 4 samples per core, no
cross-core communication.

Layout: partition p holds the channel pair {2p, 2p+1}; each sample is
one contiguous 32 KiB run per partition.

DMA: one HWDGE ring issuing back-to-back transfers reaches ~400 GB/s
and drains FIFO, so input is 5 queued transfers on sync alone
(samples 0-2 whole, sample 3 split into channel planes so its norm
work can start before the final 2 MiB lands). Outputs go out on
gpsimd/SWDGE, which doesn't contend with the sync ring's sequencing.

Per-core dataflow (samples s = 0..3):
  sync   : DMA x[s] -> samp[s%3]  (back-to-back, per-DMA semaphores)
  scalar : all norms (Square activation + accum_out); sample 3 norms
           are per-plane (8 half-size ops) to chase the tail
  vector : mask: reduce_max(psum) + is_equal  (sample 3 adds a
           plane-pair add first); select: c = V0*m0; c = (V1*m1)+c;
           c = (V2*m2)+c; obuf = (V3*m3)+c   (scalar_tensor_tensor)
  tensor : ones[128,128] @ norms -> psum (channel reduce + broadcast)
  gpsimd : memset ones; DMA obuf[s%2] -> out[s]

The argmax is realized as mask_k = (norm_k == max_k norm_k) in {0,1},
then O = sum_k mask_k * V_k. Exact float ties between component norms
(sums of ~1M random squares) are probability-zero.

Synchronization: engines are pipelined, so same-engine data deps need
semaphore handshakes (writes land at DRAIN). Every compute op on
scalar/vector increments its engine chain sem (sch/vch) and waits for
all previously-emitted ops on that engine; cross-engine waits
reference chain thresholds from the static emission plan below. Every
in-flight DMA gets its own semaphore (completions across DMAs are
not ordered by the semaphore protocol).
"""

from contextlib import ExitStack

import numpy as np

import concourse.bass as bass
from concourse import mybir
from concourse.bass_utils import run_bass_kernel_spmd

F32 = mybir.dt.float32
AX = mybir.AxisListType
OP = mybir.AluOpType
ACT = mybir.ActivationFunctionType

B, C, H, W = 32, 256, 64, 64
NCORES = 8
SPC = B // NCORES          # samples per core
H2, W2 = H // 2, W // 2    # 32, 32
SP = H * W                 # 4096 spatial elems per channel
OSP = H2 * W2              # 1024

N_SAMP_BUFS = 4
N_OBUFS = 2
LAST = SPC - 1

# ---- static emission plan ------------------------------------------------
# scalar: samples 0-2: 4 whole-sample Square+accum; sample 3: 2 groups of
# 4 half-size ops (one per channel plane)
SC_ORDER = [("n", 0), ("n", 1), ("n", 2), ("nh0", LAST), ("nh1", LAST)]
SC_SIZES = {"n": 4, "nh0": 4, "nh1": 4}
# vector: mk = mask ops (2; sample 3 has 3) ; ch = 4 chain ops
VE_ORDER = [("mk", 0), ("ch", 0), ("mk", 1), ("ch", 1), ("mk", 2),
            ("ch", 2), ("mk", 3), ("ch", 3)]
VE_SIZES = {("mk", s): 2 for s in range(SPC)}
VE_SIZES.update({("ch", s): 4 for s in range(SPC)})


def _plan(order, sizes):
    done, start, c = {}, {}, 0
    for key in order:
        start[key] = c
        c += sizes[key] if key in sizes else sizes[key[0]]
        done[key] = c
    return done, start, c


SC_DONE, SC_START, SC_TOTAL = _plan(SC_ORDER, SC_SIZES)
VE_DONE, VE_START, VE_TOTAL = _plan(VE_ORDER, VE_SIZES)


def build_nc():
    nc = bass.Bass("TRN2", target_bir_lowering=False, debug=False)
    x = nc.dram_tensor("x", [SPC, C, H, W], F32, kind="ExternalInput")
    out = nc.dram_tensor("out", [SPC, C, H2, W2], F32, kind="ExternalOutput")

    # x[s] as [128, 2, 4096]: partition p <- channel pair {2p, 2p+1}
    x_aps = [
        x.ap()[s].rearrange("(p c) h w -> p c (h w)", c=2) for s in range(SPC)
    ]
    out_aps = [
        out.ap()[s].rearrange("(p c) a b -> p c (a b)", c=2) for s in range(SPC)
    ]

    with ExitStack() as ctx:
        block = ctx.enter_context(nc.Block())
        sem = lambda name: ctx.enter_context(nc.semaphore(name))
        sb = lambda name, shape: ctx.enter_context(nc.sbuf_tensor(name, shape, F32))
        # in-DMA sems: s0..s2 whole; s3 plane0 / plane1
        dmains = [sem(f"dmain{i}") for i in range(SPC + 1)]
        dmaouts = [sem(f"dmaout{i}") for i in range(N_OBUFS)]
        g1, sch, vch, mm = sem("g1"), sem("sch"), sem("vch"), sem("mm")
        samps = [sb(f"samp{i}", [128, 2, SP]) for i in range(N_SAMP_BUFS)]
        obufs = [sb(f"obuf{i}", [128, 2, OSP]) for i in range(N_OBUFS)]
        cb1 = sb("cb1", [128, 2, OSP])
        cb2 = sb("cb2", [128, 2, OSP])
        sqs = sb("sqs", [128, 2, OSP])
        # norms: cols 4s+k for s<3 ; cols 16+k (plane0), 20+k (plane1) for s3
        norms = sb("norms", [128, 24])
        mask = sb("mask", [128, 4 * SPC])
        mx = sb("mx", [128, SPC])
        ones = sb("ones", [128, 128])
        psums = [
            ctx.enter_context(nc.psum_tensor(f"ps{i}", [128, 4], F32))
            for i in range(2)
        ]
        ps3 = ctx.enter_context(nc.psum_tensor("ps3", [128, 4], F32))

        def V(s, k, plane=None):
            i, j = divmod(k, 2)
            v6 = samps[s % N_SAMP_BUFS].ap().rearrange(
                "p c (r i q j) -> p c r i q j", r=H2, i=2, q=W2, j=2
            )
            if plane is None:
                return v6[:, :, :, i, :, j]
            return v6[:, plane, :, i, :, j]

        sq_view = lambda t: t.ap().rearrange("p c (r q) -> p c r q", r=H2)
        sq_half = lambda t: t.ap().rearrange("p c (r q) -> p c r q", r=H2)[:, 0]
        ncol = lambda c_: norms.ap()[:, c_ : c_ + 1]
        mcol = lambda s, k: mask.ap()[:, 4 * s + k : 4 * s + k + 1]

        def samp_war_waits(eng, s):
            """Before DMA-writing samp[s%3]: consumers of sample s-3 done."""
            if s >= N_SAMP_BUFS:
                sp = s - N_SAMP_BUFS
                eng.wait_ge(sch, SC_DONE[("n", sp)])
                eng.wait_ge(vch, VE_DONE[("ch", sp)])

        @block.sync
        def _(sync):
            for s in range(SPC - 1):
                samp_war_waits(sync, s)
                sync.dma_start(out=samps[s % N_SAMP_BUFS].ap(), in_=x_aps[s]).then_inc(
                    dmains[s], 16
                )
            samp_war_waits(sync, LAST)
            for plane in range(2):
                sync.dma_start(
                    out=samps[LAST % N_SAMP_BUFS].ap()[:, plane],
                    in_=x_aps[LAST][:, plane],
                ).then_inc(dmains[LAST + plane], 16)

        @block.gpsimd
        def _(gpsimd):
            gpsimd.memset(ones.ap(), 1.0).then_inc(g1, 1)
            for s in range(SPC):
                gpsimd.wait_ge(vch, VE_DONE[("ch", s)])
                if s >= N_OBUFS:
                    gpsimd.wait_ge(dmaouts[s % N_OBUFS], 16 * (s // N_OBUFS))
                gpsimd.dma_start(out=out_aps[s], in_=obufs[s % N_OBUFS].ap()).then_inc(
                    dmaouts[s % N_OBUFS], 16
                )

        @block.tensor
        def _(tensor):
            tensor.wait_ge(g1, 1)  # ones ready
            for s in range(SPC):
                if s < LAST:
                    tensor.wait_ge(sch, SC_DONE[("n", s)])
                    if s >= 2:
                        tensor.wait_ge(vch, VE_DONE[("mk", s - 2)])
                    tensor.matmul(
                        psums[s % 2].ap(),
                        ones.ap(),
                        norms.ap()[:, 4 * s : 4 * s + 4],
                        start=True,
                        stop=True,
                    ).then_inc(mm, 1)
                else:
                    # accumulate the two plane-groups in PSUM
                    tensor.wait_ge(sch, SC_DONE[("nh0", LAST)])
                    tensor.matmul(
                        ps3.ap(),
                        ones.ap(),
                        norms.ap()[:, 16:20],
                        start=True,
                        stop=False,
                    )
                    tensor.wait_ge(sch, SC_DONE[("nh1", LAST)])
                    tensor.matmul(
                        ps3.ap(),
                        ones.ap(),
                        norms.ap()[:, 20:24],
                        start=False,
                        stop=True,
                    ).then_inc(mm, 1)

        @block.scalar
        def _(scalar):
            cnt = [0]

            def emit(inst):
                inst.then_inc(sch, 1)
                cnt[0] += 1

            def barrier():
                if cnt[0]:
                    scalar.wait_ge(sch, cnt[0])

            for g, s in SC_ORDER:
                if g == "n":
                    scalar.wait_ge(dmains[s], 16)
                    for k in range(4):
                        barrier()
                        emit(
                            scalar.activation(
                                sq_view(sqs), V(s, k), ACT.Square,
                                accum_out=ncol(4 * s + k),
                            )
                        )
                else:
                    plane = 0 if g == "nh0" else 1
                    scalar.wait_ge(dmains[LAST + plane], 16)
                    for k in range(4):
                        barrier()
                        emit(
                            scalar.activation(
                                sq_half(sqs), V(s, k, plane=plane), ACT.Square,
                                accum_out=ncol(16 + 4 * plane + k),
                            )
                        )
            assert cnt[0] == SC_TOTAL

        @block.vector
        def _(vector):
            cnt = [0]

            def emit(inst):
                inst.then_inc(vch, 1)
                cnt[0] += 1

            def barrier():
                if cnt[0]:
                    vector.wait_ge(vch, cnt[0])

            def mk(s):
                vector.wait_ge(mm, s + 1)
                src = ps3.ap() if s == LAST else psums[s % 2].ap()
                barrier()
                emit(vector.reduce_max(mx.ap()[:, s : s + 1], src, axis=AX.X))
                barrier()
                emit(
                    vector.tensor_scalar(
                        out=mask.ap()[:, 4 * s : 4 * s + 4],
                        in0=src,
                        scalar1=mx.ap()[:, s : s + 1],
                        scalar2=None,
                        op0=OP.is_equal,
                    )
                )

            def ch(s):
                barrier()
                emit(vector.tensor_scalar_mul(sq_view(cb1), V(s, 0), mcol(s, 0)))
                barrier()
                emit(
                    vector.scalar_tensor_tensor(
                        out=cb2.ap(), in0=V(s, 1), scalar=mcol(s, 1), in1=cb1.ap(),
                        op0=OP.mult, op1=OP.add,
                    )
                )
                barrier()
                emit(
                    vector.scalar_tensor_tensor(
                        out=cb1.ap(), in0=V(s, 2), scalar=mcol(s, 2), in1=cb2.ap(),
                        op0=OP.mult, op1=OP.add,
                    )
                )
                if s >= N_OBUFS:
                    vector.wait_ge(dmaouts[s % N_OBUFS], 16 * (s // N_OBUFS))
                barrier()
                emit(
                    vector.scalar_tensor_tensor(
                        out=obufs[s % N_OBUFS].ap(), in0=V(s, 3), scalar=mcol(s, 3),
                        in1=cb1.ap(), op0=OP.mult, op1=OP.add,
                    )
                )

            for g, s in VE_ORDER:
                mk(s) if g == "mk" else ch(s)
            assert cnt[0] == VE_TOTAL

    return nc


_NC_CACHE = None


def _get_nc():
    global _NC_CACHE
    if _NC_CACHE is None:
        _NC_CACHE = build_nc()
    return _NC_CACHE


def kernel(x) -> np.ndarray:
    x = np.asarray(x, dtype=np.float32)
    assert x.shape == (B, C, H, W), x.shape
    shards = np.split(x, NCORES, axis=0)
    in_maps = [{"x": s} for s in shards]
    res = run_bass_kernel_spmd(_get_nc(), in_maps, core_ids=list(range(NCORES)))
    return np.concatenate([r["out"] for r in res.results], axis=0)


# revision 13
# speedup vs baseline: 1.3312x; 1.0155x over previous
"""Adaptive polyphase sampling (stride 2, p=2) on 8 TRN2 NeuronCores.

For x [32, 256, 64, 64] f32: compute the 4 polyphase components
x[:, :, i::2, j::2], pick per-sample the component with the largest L2
norm (over channels+space), return it [32, 256, 32, 32].

Sharding: pure data parallel over batch — 4 samples per core, no
cross-core communication.

Layout: partition p holds the channel pair {2p, 2p+1}; each sample is
one contiguous 32 KiB run per partition.

DMA: one HWDGE ring issuing back-to-back transfers reaches ~400 GB/s
and drains FIFO, so ALL data movement lives on the sync ring: 4 input
transfers queued immediately, then the 4 output transfers (their
descriptors enqueue when each result is ready and drain after the
inputs — no bandwidth contention during the input phase).

Per-core dataflow (samples s = 0..3):
  sync   : DMA x[s] -> samp[s] (back-to-back); DMA obuf[s%3] -> out[s]
  scalar : norms k=0..3 (Square activation + accum_out); sample 3 only
           k=0,1 (k=2,3 go to vector to shorten the tail); Square
           activation table preloaded via a zero-scale dummy op
  vector : sample-3 norms k=2,3 (scalar_tensor_tensor square+accum);
           mask: reduce_max(psum) + is_equal -> mask[:, 4s:4s+4];
           select: c = V0*m0; c = (V1*m1)+c; c = (V2*m2)+c;
           obuf = (V3*m3)+c                  (scalar_tensor_tensor)
  tensor : ones[128,128] @ norms -> psum (channel reduce + broadcast)
  gpsimd : memset ones (then idle; Block(no_gpsimd_drain) skips its
           expensive end-of-kernel DGE drain)

The argmax is realized as mask_k = (norm_k == max_k norm_k) in {0,1},
then O = sum_k mask_k * V_k. Exact float ties between component norms
(sums of ~1M random squares) are probability-zero.

Synchronization: engines are pipelined, so same-engine data deps need
semaphore handshakes (writes land at DRAIN). Every compute op on
scalar/vector increments its engine chain sem (sch/vch) and waits for
all previously-emitted ops on that engine; cross-engine waits
reference chain thresholds from the static emission plan below. Every
in-flight DMA gets its own semaphore (completions across DMAs are not
ordered by the semaphore protocol).
"""

from contextlib import ExitStack

import numpy as np

import concourse.bass as bass
from concourse import mybir
from concourse.bass_utils import run_bass_kernel_spmd

F32 = mybir.dt.float32
AX = mybir.AxisListType
OP = mybir.AluOpType
ACT = mybir.ActivationFunctionType

B, C, H, W = 32, 256, 64, 64
NCORES = 8
SPC = B // NCORES          # samples per core
H2, W2 = H // 2, W // 2    # 32, 32
SP = H * W                 # 4096 spatial elems per channel
OSP = H2 * W2              # 1024

N_SAMP_BUFS = 4
N_OBUFS = 3
LAST = SPC - 1

# ---- static emission plan ------------------------------------------------
# scalar: op 0 is the table-preload dummy; sample LAST has 2 norm ops
SC_ORDER = [("pre", 0), ("n", 0), ("n", 1), ("n", 2), ("n", LAST)]
SC_SIZES = {("pre", 0): 1, ("n", 0): 4, ("n", 1): 4, ("n", 2): 4, ("n", LAST): 2}
# vector: mk = 2 mask ops ; ch = 4 chain ops ; vn = 2 norms (sample LAST)
VE_ORDER = [("mk", 0), ("ch", 0), ("mk", 1), ("ch", 1), ("mk", 2),
            ("vn", LAST), ("ch", 2), ("mk", 3), ("ch", 3)]
VE_SIZES = {"mk": 2, "ch": 4, "vn": 2}


def _plan(order, sizes):
    done, start, c = {}, {}, 0
    for key in order:
        start[key] = c
        c += sizes[key] if key in sizes else sizes[key[0]]
        done[key] = c
    return done, start, c


SC_DONE, SC_START, SC_TOTAL = _plan(SC_ORDER, SC_SIZES)
VE_DONE, VE_START, VE_TOTAL = _plan(VE_ORDER, VE_SIZES)


def build_nc():
    nc = bass.Bass("TRN2", target_bir_lowering=False, debug=False)
    x = nc.dram_tensor("x", [SPC, C, H, W], F32, kind="ExternalInput")
    out = nc.dram_tensor("out", [SPC, C, H2, W2], F32, kind="ExternalOutput")

    # x[s] as [128, 2, 4096]: partition p <- channel pair {2p, 2p+1}
    x_aps = [
        x.ap()[s].rearrange("(p c) h w -> p c (h w)", c=2) for s in range(SPC)
    ]
    out_aps = [
        out.ap()[s].rearrange("(p c) a b -> p c (a b)", c=2) for s in range(SPC)
    ]

    with ExitStack() as ctx:
        block = ctx.enter_context(nc.Block(no_gpsimd_drain=True))
        sem = lambda name: ctx.enter_context(nc.semaphore(name))
        sb = lambda name, shape: ctx.enter_context(nc.sbuf_tensor(name, shape, F32))
        dmains = [sem(f"dmain{i}") for i in range(SPC)]
        dmaouts = [sem(f"dmaout{i}") for i in range(SPC)]
        g1, sch, vch, mm = sem("g1"), sem("sch"), sem("vch"), sem("mm")
        samps = [sb(f"samp{i}", [128, 2, SP]) for i in range(N_SAMP_BUFS)]
        obufs = [sb(f"obuf{i}", [128, 2, OSP]) for i in range(N_OBUFS)]
        cb1 = sb("cb1", [128, 2, OSP])
        cb2 = sb("cb2", [128, 2, OSP])
        sqs = sb("sqs", [128, 2, OSP])
        sqv = sb("sqv", [128, 2, OSP])
        norms = sb("norms", [128, 4 * SPC])
        mask = sb("mask", [128, 4 * SPC])
        mx = sb("mx", [128, SPC])
        ones = sb("ones", [128, 128])
        psums = [
            ctx.enter_context(nc.psum_tensor(f"ps{i}", [128, 4], F32))
            for i in range(2)
        ]

        def V(s, k):
            i, j = divmod(k, 2)
            v6 = samps[s % N_SAMP_BUFS].ap().rearrange(
                "p c (r i q j) -> p c r i q j", r=H2, i=2, q=W2, j=2
            )
            return v6[:, :, :, i, :, j]

        sq_view = lambda t: t.ap().rearrange("p c (r q) -> p c r q", r=H2)
        ncol = lambda s, k: norms.ap()[:, 4 * s + k : 4 * s + k + 1]
        mcol = lambda s, k: mask.ap()[:, 4 * s + k : 4 * s + k + 1]

        @block.sync
        def _(sync):
            for s in range(SPC):
                sync.dma_start(out=samps[s].ap(), in_=x_aps[s]).then_inc(
                    dmains[s], 16
                )
            for s in range(SPC):
                sync.wait_ge(vch, VE_DONE[("ch", s)])
                sync.dma_start(out=out_aps[s], in_=obufs[s % N_OBUFS].ap()).then_inc(
                    dmaouts[s], 16
                )
            # all outputs must land before the kernel may retire
            for s in range(SPC):
                sync.wait_ge(dmaouts[s], 16)

        @block.gpsimd
        def _(gpsimd):
            gpsimd.memset(ones.ap(), 1.0).then_inc(g1, 1)

        @block.tensor
        def _(tensor):
            tensor.wait_ge(g1, 1)  # ones ready
            for s in range(SPC):
                tensor.wait_ge(sch, SC_DONE[("n", s)])
                if s == LAST:
                    tensor.wait_ge(vch, VE_DONE[("vn", LAST)])
                if s >= 2:
                    tensor.wait_ge(vch, VE_DONE[("mk", s - 2)])
                tensor.matmul(
                    psums[s % 2].ap(),
                    ones.ap(),
                    norms.ap()[:, 4 * s : 4 * s + 4],
                    start=True,
                    stop=True,
                ).then_inc(mm, 1)

        @block.scalar
        def _(scalar):
            cnt = [0]

            def emit(inst):
                inst.then_inc(sch, 1)
                cnt[0] += 1

            def barrier():
                if cnt[0]:
                    scalar.wait_ge(sch, cnt[0])

            # preload the Square activation table before any data arrives
            # (scale=0.0 makes the engine skip reading the input)
            emit(
                scalar.activation(
                    sqs.ap()[:, 0, 0:1], sqs.ap()[:, 0, 0:1], ACT.Square, scale=0.0
                )
            )

            for g, s in SC_ORDER[1:]:
                scalar.wait_ge(dmains[s], 16)
                ks = (0, 1) if s == LAST else (0, 1, 2, 3)
                for k in ks:
                    barrier()
                    emit(
                        scalar.activation(
                            sq_view(sqs), V(s, k), ACT.Square, accum_out=ncol(s, k)
                        )
                    )
            assert cnt[0] == SC_TOTAL

        @block.vector
        def _(vector):
            cnt = [0]

            def emit(inst):
                inst.then_inc(vch, 1)
                cnt[0] += 1

            def barrier():
                if cnt[0]:
                    vector.wait_ge(vch, cnt[0])

            def vn(s):
                vector.wait_ge(dmains[s], 16)
                for k in (2, 3):
                    barrier()
                    emit(
                        vector.scalar_tensor_tensor(
                            out=sq_view(sqv),
                            in0=V(s, k),
                            scalar=0.0,
                            in1=V(s, k),
                            op0=OP.bypass,
                            op1=OP.mult,
                            accum_out=ncol(s, k),
                        )
                    )

            def mk(s):
                vector.wait_ge(mm, s + 1)
                barrier()
                emit(
                    vector.reduce_max(
                        mx.ap()[:, s : s + 1], psums[s % 2].ap(), axis=AX.X
                    )
                )
                barrier()
                emit(
                    vector.tensor_scalar(
                        out=mask.ap()[:, 4 * s : 4 * s + 4],
                        in0=psums[s % 2].ap(),
                        scalar1=mx.ap()[:, s : s + 1],
                        scalar2=None,
                        op0=OP.is_equal,
                    )
                )

            def ch(s):
                barrier()
                emit(vector.tensor_scalar_mul(sq_view(cb1), V(s, 0), mcol(s, 0)))
                barrier()
                emit(
                    vector.scalar_tensor_tensor(
                        out=cb2.ap(), in0=V(s, 1), scalar=mcol(s, 1), in1=cb1.ap(),
                        op0=OP.mult, op1=OP.add,
                    )
                )
                barrier()
                emit(
                    vector.scalar_tensor_tensor(
                        out=cb1.ap(), in0=V(s, 2), scalar=mcol(s, 2), in1=cb2.ap(),
                        op0=OP.mult, op1=OP.add,
                    )
                )
                if s >= N_OBUFS:
                    vector.wait_ge(dmaouts[s - N_OBUFS], 16)
                barrier()
                emit(
                    vector.scalar_tensor_tensor(
                        out=obufs[s % N_OBUFS].ap(), in0=V(s, 3), scalar=mcol(s, 3),
                        in1=cb1.ap(), op0=OP.mult, op1=OP.add,
                    )
                )

            fns = {"mk": mk, "ch": ch, "vn": vn}
            for g, s in VE_ORDER:
                fns[g](s)
            assert cnt[0] == VE_TOTAL

    return nc


_NC_CACHE = None


def _get_nc():
    global _NC_CACHE
    if _NC_CACHE is None:
        _NC_CACHE = build_nc()
    return _NC_CACHE


def kernel(x) -> np.ndarray:
    x = np.asarray(x, dtype=np.float32)
    assert x.shape == (B, C, H, W), x.shape
    shards = np.split(x, NCORES, axis=0)
    in_maps = [{"x": s} for s in shards]
    res = run_bass_kernel_spmd(_get_nc(), in_maps, core_ids=list(range(NCORES)))
    return np.concatenate([r["out"] for r in res.results], axis=0)


# revision 15
# speedup vs baseline: 1.3728x; 1.0312x over previous
"""Adaptive polyphase sampling (stride 2, p=2) on 8 TRN2 NeuronCores.

For x [32, 256, 64, 64] f32: compute the 4 polyphase components
x[:, :, i::2, j::2], pick per-sample the component with the largest L2
norm (over channels+space), return it [32, 256, 32, 32].

Sharding: pure data parallel over batch — 4 samples per core, no
cross-core communication.

Layout: partition p holds the channel pair {2p, 2p+1}; each sample is
one contiguous 32 KiB run per partition.

DMA: one HWDGE ring issuing back-to-back transfers reaches ~400 GB/s
and drains FIFO, so ALL data movement lives on the sync ring: 4 input
transfers queued immediately, then the 4 output transfers (their
descriptors enqueue when each result is ready and drain after the
inputs — no bandwidth contention during the input phase).

Per-core dataflow (samples s = 0..3):
  sync   : DMA x[s] -> samp[s] (back-to-back); DMA obuf[s%3] -> out[s]
  scalar : norms k=0..3 (Square activation + accum_out); sample 3 only
           k=0,1 (k=2,3 go to vector to shorten the tail); Square
           activation table preloaded via a zero-scale dummy op
  vector : sample-3 norms k=2,3 (scalar_tensor_tensor square+accum);
           mask: reduce_max(psum) + is_equal -> mask[:, 4s:4s+4];
           select: c = V0*m0; c = (V1*m1)+c; c = (V2*m2)+c;
           obuf = (V3*m3)+c                  (scalar_tensor_tensor)
  tensor : ones[128,128] @ norms -> psum (channel reduce + broadcast)
  gpsimd : memset ones (then idle; Block(no_gpsimd_drain) skips its
           expensive end-of-kernel DGE drain)

The argmax is realized as mask_k = (norm_k == max_k norm_k) in {0,1},
then O = sum_k mask_k * V_k. Exact float ties between component norms
(sums of ~1M random squares) are probability-zero.

Synchronization: engines are pipelined, so same-engine data deps need
semaphore handshakes (writes land at DRAIN). Every compute op on
scalar/vector increments its engine chain sem (sch/vch) and waits for
all previously-emitted ops on that engine; cross-engine waits
reference chain thresholds from the static emission plan below. Every
in-flight DMA gets its own semaphore (completions across DMAs are not
ordered by the semaphore protocol).
"""

from contextlib import ExitStack

import numpy as np

import concourse.bass as bass
from concourse import mybir
from concourse.bass_utils import run_bass_kernel_spmd

F32 = mybir.dt.float32
AX = mybir.AxisListType
OP = mybir.AluOpType
ACT = mybir.ActivationFunctionType

B, C, H, W = 32, 256, 64, 64
NCORES = 8
SPC = B // NCORES          # samples per core
H2, W2 = H // 2, W // 2    # 32, 32
SP = H * W                 # 4096 spatial elems per channel
OSP = H2 * W2              # 1024

N_SAMP_BUFS = 4
N_OBUFS = 3
LAST = SPC - 1

# ---- static emission plan ------------------------------------------------
# scalar: op 0 is the table-preload dummy; 4 norms per sample
SC_ORDER = [("pre", 0)] + [("n", s) for s in range(SPC)]
SC_SIZES = {"pre": 1, "n": 4}
# vector: mk = 2 mask ops ; ch = 4 chain ops
VE_ORDER = [("mk", 0), ("ch", 0), ("mk", 1), ("ch", 1), ("mk", 2),
            ("ch", 2), ("mk", 3), ("ch", 3)]
VE_SIZES = {"mk": 2, "ch": 4}
# gpsimd: just the ones-memset
GP_ORDER = [("ones", 0)]
GP_SIZES = {"ones": 1}


def _plan(order, sizes):
    done, start, c = {}, {}, 0
    for key in order:
        start[key] = c
        c += sizes[key] if key in sizes else sizes[key[0]]
        done[key] = c
    return done, start, c


SC_DONE, SC_START, SC_TOTAL = _plan(SC_ORDER, SC_SIZES)
VE_DONE, VE_START, VE_TOTAL = _plan(VE_ORDER, VE_SIZES)
GP_DONE, GP_START, GP_TOTAL = _plan(GP_ORDER, GP_SIZES)


def build_nc():
    nc = bass.Bass("TRN2", target_bir_lowering=False, debug=False)
    x = nc.dram_tensor("x", [SPC, C, H, W], F32, kind="ExternalInput")
    out = nc.dram_tensor("out", [SPC, C, H2, W2], F32, kind="ExternalOutput")

    # x[s] as [128, 2, 4096]: partition p <- channel pair {2p, 2p+1}
    x_aps = [
        x.ap()[s].rearrange("(p c) h w -> p c (h w)", c=2) for s in range(SPC)
    ]
    out_aps = [
        out.ap()[s].rearrange("(p c) a b -> p c (a b)", c=2) for s in range(SPC)
    ]

    with ExitStack() as ctx:
        block = ctx.enter_context(nc.Block(no_gpsimd_drain=True))
        sem = lambda name: ctx.enter_context(nc.semaphore(name))
        sb = lambda name, shape: ctx.enter_context(nc.sbuf_tensor(name, shape, F32))
        dmains = [sem(f"dmain{i}") for i in range(SPC)]
        dmaouts = [sem(f"dmaout{i}") for i in range(SPC)]
        sch, vch, gch, mm = sem("sch"), sem("vch"), sem("gch"), sem("mm")
        samps = [sb(f"samp{i}", [128, 2, SP]) for i in range(N_SAMP_BUFS)]
        obufs = [sb(f"obuf{i}", [128, 2, OSP]) for i in range(N_OBUFS)]
        cb1 = sb("cb1", [128, 2, OSP])
        cb2 = sb("cb2", [128, 2, OSP])
        sqs = sb("sqs", [128, 2, OSP])
        sqv = sb("sqv", [128, 2, OSP])
        norms = sb("norms", [128, 4 * SPC])
        mask = sb("mask", [128, 4 * SPC])
        mx = sb("mx", [128, SPC])
        ones = sb("ones", [128, 128])
        psums = [
            ctx.enter_context(nc.psum_tensor(f"ps{i}", [128, 4], F32))
            for i in range(2)
        ]

        def V(s, k):
            i, j = divmod(k, 2)
            v6 = samps[s % N_SAMP_BUFS].ap().rearrange(
                "p c (r i q j) -> p c r i q j", r=H2, i=2, q=W2, j=2
            )
            return v6[:, :, :, i, :, j]

        sq_view = lambda t: t.ap().rearrange("p c (r q) -> p c r q", r=H2)
        ncol = lambda s, k: norms.ap()[:, 4 * s + k : 4 * s + k + 1]
        mcol = lambda s, k: mask.ap()[:, 4 * s + k : 4 * s + k + 1]

        @block.sync
        def _(sync):
            for s in range(SPC):
                sync.dma_start(out=samps[s].ap(), in_=x_aps[s]).then_inc(
                    dmains[s], 16
                )
            sync.wait_ge(dmains[LAST], 16)
            for s in range(SPC):
                sync.wait_ge(vch, VE_DONE[("ch", s)])
                sync.dma_start(out=out_aps[s], in_=obufs[s % N_OBUFS].ap()).then_inc(
                    dmaouts[s], 16
                )
            # all outputs must land before the kernel may retire
            for s in range(SPC):
                sync.wait_ge(dmaouts[s], 16)

        @block.gpsimd
        def _(gpsimd):
            gcnt = [0]

            def gemit(inst):
                inst.then_inc(gch, 1)
                gcnt[0] += 1

            def gbarrier():
                if gcnt[0]:
                    gpsimd.wait_ge(gch, gcnt[0])

            gemit(gpsimd.memset(ones.ap(), 1.0))

        @block.tensor
        def _(tensor):
            for s in range(SPC):
                tensor.wait_ge(sch, SC_DONE[("n", s)])
                tensor.wait_ge(gch, 1)
                if s >= 2:
                    tensor.wait_ge(vch, VE_DONE[("mk", s - 2)])
                tensor.matmul(
                    psums[s % 2].ap(),
                    ones.ap(),
                    norms.ap()[:, 4 * s : 4 * s + 4],
                    start=True,
                    stop=True,
                ).then_inc(mm, 1)

        @block.scalar
        def _(scalar):
            cnt = [0]

            def emit(inst):
                inst.then_inc(sch, 1)
                cnt[0] += 1

            def barrier():
                if cnt[0]:
                    scalar.wait_ge(sch, cnt[0])

            # preload the Square activation table before any data arrives
            # (scale=0.0 makes the engine skip reading the input)
            emit(
                scalar.activation(
                    sqs.ap()[:, 0, 0:1], sqs.ap()[:, 0, 0:1], ACT.Square, scale=0.0
                )
            )

            for g, s in SC_ORDER[1:]:
                scalar.wait_ge(dmains[s], 16)
                for k in (0, 1, 2, 3):
                    barrier()
                    emit(
                        scalar.activation(
                            sq_view(sqs), V(s, k), ACT.Square, accum_out=ncol(s, k)
                        )
                    )
            assert cnt[0] == SC_TOTAL

        @block.vector
        def _(vector):
            cnt = [0]

            def emit(inst):
                inst.then_inc(vch, 1)
                cnt[0] += 1

            def barrier():
                if cnt[0]:
                    vector.wait_ge(vch, cnt[0])

            def mk(s):
                vector.wait_ge(mm, s + 1)
                barrier()
                emit(
                    vector.reduce_max(
                        mx.ap()[:, s : s + 1], psums[s % 2].ap(), axis=AX.X
                    )
                )
                barrier()
                emit(
                    vector.tensor_scalar(
                        out=mask.ap()[:, 4 * s : 4 * s + 4],
                        in0=psums[s % 2].ap(),
                        scalar1=mx.ap()[:, s : s + 1],
                        scalar2=None,
                        op0=OP.is_equal,
                    )
                )

            def ch(s):
                barrier()
                emit(vector.tensor_scalar_mul(sq_view(cb1), V(s, 0), mcol(s, 0)))
                barrier()
                emit(
                    vector.scalar_tensor_tensor(
                        out=cb2.ap(), in0=V(s, 1), scalar=mcol(s, 1), in1=cb1.ap(),
                        op0=OP.mult, op1=OP.add,
                    )
                )
                barrier()
                emit(
                    vector.scalar_tensor_tensor(
                        out=cb1.ap(), in0=V(s, 2), scalar=mcol(s, 2), in1=cb2.ap(),
                        op0=OP.mult, op1=OP.add,
                    )
                )
                if s >= N_OBUFS:
                    vector.wait_ge(dmaouts[s - N_OBUFS], 16)
                barrier()
                emit(
                    vector.scalar_tensor_tensor(
                        out=obufs[s % N_OBUFS].ap(), in0=V(s, 3), scalar=mcol(s, 3),
                        in1=cb1.ap(), op0=OP.mult, op1=OP.add,
                    )
                )

            fns = {"mk": mk, "ch": ch}
            for g, s in VE_ORDER:
                fns[g](s)
            assert cnt[0] == VE_TOTAL

    return nc


_NC_CACHE = None


def _get_nc():
    global _NC_CACHE
    if _NC_CACHE is None:
        _NC_CACHE = build_nc()
    return _NC_CACHE


def kernel(x) -> np.ndarray:
    x = np.asarray(x, dtype=np.float32)
    assert x.shape == (B, C, H, W), x.shape
    shards = np.split(x, NCORES, axis=0)
    in_maps = [{"x": s} for s in shards]
    res = run_bass_kernel_spmd(_get_nc(), in_maps, core_ids=list(range(NCORES)))
    return np.concatenate([r["out"] for r in res.results], axis=0)
